# revision 5
# baseline (speedup 1.0000x reference)
"""Trainium2 Bass kernel for nn_CustomLoss_86672440033625.

loss = cross_entropy_mean(logits, tgt) + 10.0 * lower_triangle_overlap_count(predictions)

Strategy (8 NeuronCores, SPMD, full inputs in / full output out):
  - CE term: shard the 8192 logit rows across cores (1024 rows/core, 131MB/core —
    the memory-roofline term). Per core: stream [128, VC] chunks, ACT-engine
    exp with per-instruction accumulate -> per-row sum(exp); ln on ACT.
    Target logits gathered with one indirect (DGE) DMA using host-prepared
    flat indices.
  - Overlap term: the pair predicate is symmetric in (i, j), so
    count_{j<i} = (sum over ALL ordered pairs incl. diag - #diag)/2 — no tril
    mask needed. Shard objects i across cores (256/core). Partition layout
    p = (i%4)*32 + station so the j-broadcast tiles (ends/starts of all 2048
    objects per station) are core-independent constants. Per group of 4 i's:
    two fp32 tensor_scalar compares (2x mode) + one fused
    tensor_tensor_reduce (mult+running-add) on the DVE.
  - Final combine of the 8 cores' partial scalars happens on host in f64.
"""

import os
import sys

import numpy as np


def _ensure_concourse():
    try:
        import concourse  # noqa: F401
    except ImportError:
        for p in ("/opt/trn_rl_repo", "/root/.axon_site/_ro/trn_rl_repo"):
            if os.path.isdir(p):
                sys.path.insert(0, p)
                break


N_CORES = 8
B, S, V = 4, 2048, 32000
R_TOTAL = B * S  # 8192 rows
R_CORE = R_TOTAL // N_CORES  # 1024 rows per core
RB = R_CORE // 128  # 8 row blocks per core
VC = 8000  # vocab chunk (free dim of a streamed tile)
NCH = V // VC  # 4 chunks per row block

N_OBJ, ST = 2048, 32
I_CORE = N_OBJ // N_CORES  # 256 objects per core
GROUPS = I_CORE // 4  # 64 groups of (4 objects x 32 stations) = 128 partitions

PENALTY = 10.0

LAST_EXEC_NS = None
LAST_RESULTS = None

_NC_CACHE = None


def _build_nc(enable_ce=True, enable_ov=True, enable_gat=True):
    _ensure_concourse()
    import concourse.bacc as bacc
    import concourse.bass as bass
    import concourse.mybir as mybir
    import concourse.tile as tile
    from concourse._compat import get_trn_type

    f32 = mybir.dt.float32
    bf16 = mybir.dt.bfloat16
    i32 = mybir.dt.int32
    Op = mybir.AluOpType
    AF = mybir.ActivationFunctionType

    nc = bacc.Bacc(get_trn_type() or "TRN2", target_bir_lowering=False, debug=False)

    logits_d = nc.dram_tensor("logits", [R_CORE, V], f32, kind="ExternalInput")
    idx_d = nc.dram_tensor("gidx", [128, RB], i32, kind="ExternalInput")
    sb_d = nc.dram_tensor("s_bcast", [128, N_OBJ], f32, kind="ExternalInput")
    eb_d = nc.dram_tensor("e_bcast", [128, N_OBJ], f32, kind="ExternalInput")
    scol_d = nc.dram_tensor("s_col", [128, GROUPS], f32, kind="ExternalInput")
    ecol_d = nc.dram_tensor("e_col", [128, GROUPS], f32, kind="ExternalInput")

    lse_d = nc.dram_tensor("lse_out", [128, RB], f32, kind="ExternalOutput")
    gat_d = nc.dram_tensor("gat_out", [128, RB], f32, kind="ExternalOutput")
    cnt_d = nc.dram_tensor("cnt_out", [128, 1], f32, kind="ExternalOutput")

    with tile.TileContext(nc) as tc:
        with (
            tc.tile_pool(name="const", bufs=1) as cp,
            tc.tile_pool(name="stream", bufs=4) as sp,
            tc.tile_pool(name="ov", bufs=2) as vp,
        ):
            sb = cp.tile([128, N_OBJ], f32)
            nc.sync.dma_start(out=sb[:], in_=sb_d[:])
            eb = cp.tile([128, N_OBJ], f32)
            nc.sync.dma_start(out=eb[:], in_=eb_d[:])
            scol = cp.tile([128, GROUPS], f32)
            nc.sync.dma_start(out=scol[:], in_=scol_d[:])
            ecol = cp.tile([128, GROUPS], f32)
            nc.sync.dma_start(out=ecol[:], in_=ecol_d[:])
            idxt = cp.tile([128, RB], i32)
            nc.sync.dma_start(out=idxt[:], in_=idx_d[:])

            # ---- overlap term (DVE) ----
            acc = cp.tile([128, 1], f32)
            acc_cols = cp.tile([128, GROUPS], f32)
            nc.vector.memset(acc_cols[:], 0.0)
            scr = cp.tile([128, N_OBJ], bf16)
            for g in range(GROUPS if enable_ov else 0):
                a = vp.tile([128, N_OBJ], bf16, tag="a")
                nc.vector.tensor_scalar(
                    out=a[:],
                    in0=eb[:],
                    scalar1=scol[:, g : g + 1],
                    scalar2=None,
                    op0=Op.is_gt,
                )
                b = vp.tile([128, N_OBJ], bf16, tag="b")
                nc.vector.tensor_scalar(
                    out=b[:],
                    in0=sb[:],
                    scalar1=ecol[:, g : g + 1],
                    scalar2=None,
                    op0=Op.is_lt,
                )
                m = vp.tile([128, N_OBJ], bf16, tag="m")
                nc.vector.tensor_tensor(out=m[:], in0=a[:], in1=b[:], op=Op.mult)
                # accumulate sum_j of the AND-mask into this group's column
                nc.vector.tensor_scalar(
                    out=scr[:],
                    in0=m[:],
                    scalar1=1.0,
                    scalar2=None,
                    op0=Op.mult,
                    op1=Op.add,
                    accum_out=acc_cols[:, g : g + 1],
                )
            nc.vector.tensor_reduce(
                out=acc[:],
                in_=acc_cols[:],
                axis=mybir.AxisListType.X,
                op=Op.add,
            )
            nc.sync.dma_start(out=cnt_d[:], in_=acc[:])

            # ---- CE term: sum(exp) per row via ACT accumulate ----
            sume = cp.tile([128, RB * NCH], f32)
            nc.vector.memset(sume[:], 0.0)
            for rb in range(RB if enable_ce else 0):
                for c in range(NCH):
                    t = sp.tile([128, VC], f32, tag="chunk")
                    nc.sync.dma_start(
                        out=t[:],
                        in_=logits_d[rb * 128 : (rb + 1) * 128, c * VC : (c + 1) * VC],
                    )
                    k = rb * NCH + c
                    nc.scalar.activation(
                        out=t[:],
                        in_=t[:],
                        func=AF.Exp,
                        accum_out=sume[:, k : k + 1],
                    )
            sums = cp.tile([128, RB], f32)
            nc.vector.tensor_reduce(
                out=sums[:],
                in_=sume[:].rearrange("p (b c) -> p b c", c=NCH),
                axis=mybir.AxisListType.X,
                op=Op.add,
            )
            lse = cp.tile([128, RB], f32)
            nc.scalar.activation(out=lse[:], in_=sums[:], func=AF.Ln)
            nc.sync.dma_start(out=lse_d[:], in_=lse[:])

            # ---- target-logit gather (indirect DMA) ----
            # HW semantics: ONE index per partition; each index fetches the
            # dest row's free extent. So issue RB gathers of [128, 1] each.
            gat = cp.tile([128, RB], f32)
            nc.vector.memset(gat[:], 0.0)
            if enable_gat:
                for rb in range(RB):
                    nc.gpsimd.indirect_dma_start(
                        out=gat[:, rb : rb + 1],
                        out_offset=None,
                        in_=logits_d[:],
                        in_offset=bass.IndirectOffsetOnAxis(
                            ap=idxt[:, rb : rb + 1], axis=1
                        ),
                    )
            nc.sync.dma_start(out=gat_d[:], in_=gat[:])

    nc.compile()
    return nc


def get_nc():
    global _NC_CACHE
    if _NC_CACHE is None:
        _NC_CACHE = _build_nc()
    return _NC_CACHE


def make_core_inputs(logits, tgt, predictions):
    """Host-side sharding/prep. Returns (per_core_input_maps, diag_count)."""
    logits = np.ascontiguousarray(np.asarray(logits, dtype=np.float32)).reshape(
        R_TOTAL, V
    )
    tgt = np.asarray(tgt).reshape(R_TOTAL).astype(np.int64)
    preds = np.asarray(predictions, dtype=np.float32)
    starts = preds[:, :, 0]  # [N_OBJ, ST]
    ends = (starts + preds[:, :, 1]).astype(np.float32)  # same f32 add as reference

    # partition p = (i mod 4)*32 + station ; station = p % 32, i4 = p // 32
    sb_full = np.ascontiguousarray(np.tile(starts.T, (4, 1)))  # [128, N_OBJ]
    eb_full = np.ascontiguousarray(np.tile(ends.T, (4, 1)))  # [128, N_OBJ]
    i4 = np.arange(128) // 32
    sid = np.arange(128) % 32

    per_core = []
    for c in range(N_CORES):
        base_r = c * R_CORE
        r_local = np.arange(RB)[None, :] * 128 + np.arange(128)[:, None]  # [128, RB]
        gidx = (r_local * V + tgt[base_r + r_local]).astype(np.int32)
        base_i = c * I_CORE
        obj = base_i + np.arange(GROUPS)[None, :] * 4 + i4[:, None]  # [128, GROUPS]
        s_col = np.ascontiguousarray(starts[obj, sid[:, None]].astype(np.float32))
        e_col = np.ascontiguousarray(ends[obj, sid[:, None]].astype(np.float32))
        per_core.append(
            {
                "logits": logits[base_r : base_r + R_CORE],
                "gidx": gidx,
                "s_bcast": sb_full,
                "e_bcast": eb_full,
                "s_col": s_col,
                "e_col": e_col,
            }
        )
    diag = float(np.sum(starts < ends))
    return per_core, diag


def combine_outputs(out_maps, diag):
    ce_sum = 0.0
    cnt = 0.0
    for m in out_maps:
        ce_sum += float(np.sum(np.asarray(m["lse_out"], dtype=np.float64)))
        ce_sum -= float(np.sum(np.asarray(m["gat_out"], dtype=np.float64)))
        cnt += float(np.sum(np.asarray(m["cnt_out"], dtype=np.float64)))
    count_lower = (cnt - diag) / 2.0
    loss = ce_sum / R_TOTAL + PENALTY * count_lower
    return np.float32(loss)


def kernel(logits, tgt, predictions, stations=ST, obj_spacing=1, **_unused):
    global LAST_EXEC_NS, LAST_RESULTS
    _ensure_concourse()
    from concourse.bass_utils import run_bass_kernel_spmd

    per_core, diag = make_core_inputs(logits, tgt, predictions)
    nc = get_nc()
    res = run_bass_kernel_spmd(nc, per_core, list(range(N_CORES)))
    LAST_EXEC_NS = res.exec_time_ns
    LAST_RESULTS = res
    return combine_outputs(res.results, diag)


# revision 6
# speedup vs baseline: 1.2849x; 1.2849x over previous
"""Trainium2 Bass kernel for nn_CustomLoss_86672440033625.

loss = cross_entropy_mean(logits, tgt) + 10.0 * lower_triangle_overlap_count(predictions)

Strategy (8 NeuronCores, SPMD, full inputs in / full output out):
  - CE term: shard the 8192 logit rows across cores (1024 rows/core, 131MB/core —
    the memory-roofline term). Per core: stream [128, VC] chunks, ACT-engine
    exp with per-instruction accumulate -> per-row sum(exp); ln on ACT.
    Target logits gathered with one indirect (DGE) DMA using host-prepared
    flat indices.
  - Overlap term: the pair predicate is symmetric in (i, j), so
    count_{j<i} = (sum over ALL ordered pairs incl. diag - #diag)/2 — no tril
    mask needed. Shard objects i across cores (256/core). Partition layout
    p = (i%4)*32 + station so the j-broadcast tiles (ends/starts of all 2048
    objects per station) are core-independent constants. Per group of 4 i's:
    two fp32 tensor_scalar compares (2x mode) + one fused
    tensor_tensor_reduce (mult+running-add) on the DVE.
  - Final combine of the 8 cores' partial scalars happens on host in f64.
"""

import os
import sys

import numpy as np


def _ensure_concourse():
    try:
        import concourse  # noqa: F401
    except ImportError:
        for p in ("/opt/trn_rl_repo", "/root/.axon_site/_ro/trn_rl_repo"):
            if os.path.isdir(p):
                sys.path.insert(0, p)
                break


N_CORES = 8
B, S, V = 4, 2048, 32000
R_TOTAL = B * S  # 8192 rows
R_CORE = R_TOTAL // N_CORES  # 1024 rows per core
RB = R_CORE // 128  # 8 row blocks per core
VC = 8000  # vocab chunk (free dim of a streamed tile)
NCH = V // VC  # 4 chunks per row block

N_OBJ, ST = 2048, 32
I_CORE = N_OBJ // N_CORES  # 256 objects per core
GROUPS = I_CORE // 4  # 64 groups of (4 objects x 32 stations) = 128 partitions

PENALTY = 10.0

LAST_EXEC_NS = None
LAST_RESULTS = None

_NC_CACHE = None


def _build_nc(enable_ce=True, enable_ov=True, enable_gat=True):
    _ensure_concourse()
    import concourse.bacc as bacc
    import concourse.bass as bass
    import concourse.mybir as mybir
    import concourse.tile as tile
    from concourse._compat import get_trn_type

    f32 = mybir.dt.float32
    bf16 = mybir.dt.bfloat16
    i32 = mybir.dt.int32
    Op = mybir.AluOpType
    AF = mybir.ActivationFunctionType

    nc = bacc.Bacc(get_trn_type() or "TRN2", target_bir_lowering=False, debug=False)

    logits_d = nc.dram_tensor("logits", [R_CORE, V], f32, kind="ExternalInput")
    idx_d = nc.dram_tensor("gidx", [128, RB], i32, kind="ExternalInput")
    sb_d = nc.dram_tensor("s_bcast", [128, N_OBJ], f32, kind="ExternalInput")
    eb_d = nc.dram_tensor("e_bcast", [128, N_OBJ], f32, kind="ExternalInput")
    scol_d = nc.dram_tensor("s_col", [128, GROUPS], f32, kind="ExternalInput")
    ecol_d = nc.dram_tensor("e_col", [128, GROUPS], f32, kind="ExternalInput")

    lse_d = nc.dram_tensor("lse_out", [128, RB], f32, kind="ExternalOutput")
    gat_d = nc.dram_tensor("gat_out", [128, RB], f32, kind="ExternalOutput")
    cnt_d = nc.dram_tensor("cnt_out", [128, 1], f32, kind="ExternalOutput")

    with tile.TileContext(nc) as tc:
        with (
            tc.tile_pool(name="const", bufs=1) as cp,
            tc.tile_pool(name="stream", bufs=4) as sp,
            tc.tile_pool(name="ov", bufs=2) as vp,
        ):
            sb = cp.tile([128, N_OBJ], f32)
            nc.sync.dma_start(out=sb[:], in_=sb_d[:])
            eb = cp.tile([128, N_OBJ], f32)
            nc.sync.dma_start(out=eb[:], in_=eb_d[:])
            scol = cp.tile([128, GROUPS], f32)
            nc.sync.dma_start(out=scol[:], in_=scol_d[:])
            ecol = cp.tile([128, GROUPS], f32)
            nc.sync.dma_start(out=ecol[:], in_=ecol_d[:])
            idxt = cp.tile([128, RB], i32)
            nc.sync.dma_start(out=idxt[:], in_=idx_d[:])

            # ---- CE term: sum(exp) per row via ACT accumulate ----
            sume = cp.tile([128, RB * NCH], f32)
            nc.vector.memset(sume[:], 0.0)
            for rb in range(RB if enable_ce else 0):
                for c in range(NCH):
                    t = sp.tile([128, VC], f32, tag="chunk")
                    nc.sync.dma_start(
                        out=t[:],
                        in_=logits_d[rb * 128 : (rb + 1) * 128, c * VC : (c + 1) * VC],
                    )
                    k = rb * NCH + c
                    nc.scalar.activation(
                        out=t[:],
                        in_=t[:],
                        func=AF.Exp,
                        accum_out=sume[:, k : k + 1],
                    )
            sums = cp.tile([128, RB], f32)
            nc.vector.tensor_reduce(
                out=sums[:],
                in_=sume[:].rearrange("p (b c) -> p b c", c=NCH),
                axis=mybir.AxisListType.X,
                op=Op.add,
            )
            lse = cp.tile([128, RB], f32)
            nc.scalar.activation(out=lse[:], in_=sums[:], func=AF.Ln)

            # ---- overlap term (DVE) ----
            acc = cp.tile([128, 1], f32)
            acc_cols = cp.tile([128, GROUPS], f32)
            nc.vector.memset(acc_cols[:], 0.0)
            scr = cp.tile([128, N_OBJ], bf16)
            for g in range(GROUPS if enable_ov else 0):
                a = vp.tile([128, N_OBJ], bf16, tag="a")
                nc.vector.tensor_scalar(
                    out=a[:],
                    in0=eb[:],
                    scalar1=scol[:, g : g + 1],
                    scalar2=None,
                    op0=Op.is_gt,
                )
                b = vp.tile([128, N_OBJ], bf16, tag="b")
                nc.vector.tensor_scalar(
                    out=b[:],
                    in0=sb[:],
                    scalar1=ecol[:, g : g + 1],
                    scalar2=None,
                    op0=Op.is_lt,
                )
                m = vp.tile([128, N_OBJ], bf16, tag="m")
                nc.vector.tensor_tensor(out=m[:], in0=a[:], in1=b[:], op=Op.mult)
                # accumulate sum_j of the AND-mask into this group's column
                nc.vector.tensor_scalar(
                    out=scr[:],
                    in0=m[:],
                    scalar1=1.0,
                    scalar2=None,
                    op0=Op.mult,
                    op1=Op.add,
                    accum_out=acc_cols[:, g : g + 1],
                )
            nc.vector.tensor_reduce(
                out=acc[:],
                in_=acc_cols[:],
                axis=mybir.AxisListType.X,
                op=Op.add,
            )

            # ---- target-logit gather (indirect DMA) ----
            # HW semantics: ONE index per partition; each index fetches the
            # dest row's free extent. So issue RB gathers of [128, 1] each.
            gat = cp.tile([128, RB], f32)
            nc.vector.memset(gat[:], 0.0)
            if enable_gat:
                for rb in range(RB):
                    nc.gpsimd.indirect_dma_start(
                        out=gat[:, rb : rb + 1],
                        out_offset=None,
                        in_=logits_d[:],
                        in_offset=bass.IndirectOffsetOnAxis(
                            ap=idxt[:, rb : rb + 1], axis=1
                        ),
                    )
            # ---- output DMAs (emitted last so they never block the
            # logits stream in the Sync HWDGE FIFO) ----
            nc.sync.dma_start(out=lse_d[:], in_=lse[:])
            nc.sync.dma_start(out=gat_d[:], in_=gat[:])
            nc.sync.dma_start(out=cnt_d[:], in_=acc[:])

    nc.compile()
    return nc


def get_nc():
    global _NC_CACHE
    if _NC_CACHE is None:
        _NC_CACHE = _build_nc()
    return _NC_CACHE


def make_core_inputs(logits, tgt, predictions):
    """Host-side sharding/prep. Returns (per_core_input_maps, diag_count)."""
    logits = np.ascontiguousarray(np.asarray(logits, dtype=np.float32)).reshape(
        R_TOTAL, V
    )
    tgt = np.asarray(tgt).reshape(R_TOTAL).astype(np.int64)
    preds = np.asarray(predictions, dtype=np.float32)
    starts = preds[:, :, 0]  # [N_OBJ, ST]
    ends = (starts + preds[:, :, 1]).astype(np.float32)  # same f32 add as reference

    # partition p = (i mod 4)*32 + station ; station = p % 32, i4 = p // 32
    sb_full = np.ascontiguousarray(np.tile(starts.T, (4, 1)))  # [128, N_OBJ]
    eb_full = np.ascontiguousarray(np.tile(ends.T, (4, 1)))  # [128, N_OBJ]
    i4 = np.arange(128) // 32
    sid = np.arange(128) % 32

    per_core = []
    for c in range(N_CORES):
        base_r = c * R_CORE
        r_local = np.arange(RB)[None, :] * 128 + np.arange(128)[:, None]  # [128, RB]
        gidx = (r_local * V + tgt[base_r + r_local]).astype(np.int32)
        base_i = c * I_CORE
        obj = base_i + np.arange(GROUPS)[None, :] * 4 + i4[:, None]  # [128, GROUPS]
        s_col = np.ascontiguousarray(starts[obj, sid[:, None]].astype(np.float32))
        e_col = np.ascontiguousarray(ends[obj, sid[:, None]].astype(np.float32))
        per_core.append(
            {
                "logits": logits[base_r : base_r + R_CORE],
                "gidx": gidx,
                "s_bcast": sb_full,
                "e_bcast": eb_full,
                "s_col": s_col,
                "e_col": e_col,
            }
        )
    diag = float(np.sum(starts < ends))
    return per_core, diag


def combine_outputs(out_maps, diag):
    ce_sum = 0.0
    cnt = 0.0
    for m in out_maps:
        ce_sum += float(np.sum(np.asarray(m["lse_out"], dtype=np.float64)))
        ce_sum -= float(np.sum(np.asarray(m["gat_out"], dtype=np.float64)))
        cnt += float(np.sum(np.asarray(m["cnt_out"], dtype=np.float64)))
    count_lower = (cnt - diag) / 2.0
    loss = ce_sum / R_TOTAL + PENALTY * count_lower
    return np.float32(loss)


def kernel(logits, tgt, predictions, stations=ST, obj_spacing=1, **_unused):
    global LAST_EXEC_NS, LAST_RESULTS
    _ensure_concourse()
    from concourse.bass_utils import run_bass_kernel_spmd

    per_core, diag = make_core_inputs(logits, tgt, predictions)
    nc = get_nc()
    res = run_bass_kernel_spmd(nc, per_core, list(range(N_CORES)))
    LAST_EXEC_NS = res.exec_time_ns
    LAST_RESULTS = res
    return combine_outputs(res.results, diag)
